# revision 44
# baseline (speedup 1.0000x reference)
"""Distributed Trainium2 Bass kernel for the AttentionBlock problem.

Math (per batch b):
  q/k/v = x @ W + b ; scores = (q.k^T)/8 + pos[b,k,h], masked -> -inf,
  dummy col 0 ; pattern = softmax ; out = LayerNorm((pattern @ v) @ W_O)

Device-side structure (v2.1 — fine-grained interleave):
  * 8 cores, no collectives: the 4096 (b, seq) rows are split 512/core for
    the q path; each core redundantly computes its batch's FULL k/v
    projections (replication beats a slow AllGather on this fabric).
  * K/V projection matmuls are chopped into ~8-matmul chunks and emitted
    INSIDE the attention kt-loop of the previous head pair, so the
    in-order PE queue always has independent work while score banks wait
    on ScalarE's exp.
  * Scores are row-tiled: head 2j contracts over partitions 0:64 (row
    groups 0-1), head 2j+1 over 64:128 (groups 2-3) — concurrent.
  * Softmax bias (additive pos + mask) is folded multiplicatively:
    host precomputes exp_b = mask * exp(pos); v' = v * exp_b and the
    denominator column holds exp_b, so exp() needs NO bias and one
    ACTIVATE covers a [128,1024] two-bank PSUM tile.  b_V folds into a
    host-precomputed out-proj constant.  The softmax denominator
    reciprocal is computed 64-lane-wide via a [1,512]->[64,8] DMA
    reshape (DVE reciprocal is ~5ns/elem/lane).
"""

import os
from contextlib import ExitStack

import numpy as np

import concourse.bass as bass
import concourse.tile as tile
from concourse import bacc, mybir
from concourse.bass_utils import run_bass_kernel_spmd

B, SQ, SK = 2, 2048, 2048
D = 1024  # QD == KD
H, HS = 16, 64
ED = 1024
NCORES = 8
RPC = B * SQ // NCORES  # 512 query rows per core
NKT = SK // 128  # 16 kpos tiles per batch
NDT = D // 128  # 8 contraction tiles
NOT = (H * HS) // 128  # 8 head pairs
NRT = RPC // 128  # 4 row tiles per core
NRB = SK // 512  # 4 row blocks per batch
GROUP = NCORES // B  # 4 cores per batch

F32 = mybir.dt.float32
BF16 = mybir.dt.bfloat16
FP8 = mybir.dt.float8e4
DR = mybir.MatmulPerfMode.DoubleRow
WSCALE = 32.0  # host multiplies W_Q/W_K by this before fp8 cast
AF = mybir.ActivationFunctionType
ALU = mybir.AluOpType

LN_EPS = 1e-5

LAST_EXEC_NS = None

_CACHED = {}


def _build():
    nc = bacc.Bacc(None, target_bir_lowering=False)

    xqt = nc.dram_tensor("xqt", [D, RPC], BF16, kind="ExternalInput")
    xkt = nc.dram_tensor("xkt", [D, SK], BF16, kind="ExternalInput")
    xvt = nc.dram_tensor("xvt", [D, SK], BF16, kind="ExternalInput")
    wq = nc.dram_tensor("wq", [D, H * HS], BF16, kind="ExternalInput")
    wk = nc.dram_tensor("wk", [D, H * HS], BF16, kind="ExternalInput")
    wv = nc.dram_tensor("wv", [D, H * HS], BF16, kind="ExternalInput")
    wo = nc.dram_tensor("wo", [H * HS, ED], BF16, kind="ExternalInput")
    bq = nc.dram_tensor("bq", [128, NOT], F32, kind="ExternalInput")
    bk = nc.dram_tensor("bk", [128, NOT], F32, kind="ExternalInput")
    expb = nc.dram_tensor("expb", [128, NKT * H], F32, kind="ExternalInput")
    bo = nc.dram_tensor("bo", [1, ED], F32, kind="ExternalInput")
    lng = nc.dram_tensor("lng", [1, ED], BF16, kind="ExternalInput")
    lnb = nc.dram_tensor("lnb", [1, ED], BF16, kind="ExternalInput")
    out = nc.dram_tensor("out", [RPC, ED], F32, kind="ExternalOutput")
    # DRAM bounce buffer for the denominator partition-broadcast (DMA can
    # partition-broadcast from DRAM but not from SBUF)
    dscr = nc.dram_tensor("dscr", [NOT * 2, RPC], BF16, kind="Internal")

    with tile.TileContext(nc) as tc, ExitStack() as ctx:
        consts = ctx.enter_context(tc.tile_pool(name="consts", bufs=1))
        kvres = ctx.enter_context(tc.tile_pool(name="kvres", bufs=1))
        wqp = ctx.enter_context(tc.tile_pool(name="wqp", bufs=8))
        wkp = ctx.enter_context(tc.tile_pool(name="wkp", bufs=8))
        wvp = ctx.enter_context(tc.tile_pool(name="wvp", bufs=8))
        evac = ctx.enter_context(tc.tile_pool(name="evac", bufs=2))
        qzpool = ctx.enter_context(tc.tile_pool(name="qzpool", bufs=1))
        ppool = ctx.enter_context(tc.tile_pool(name="ppool", bufs=2))
        ypool = ctx.enter_context(tc.tile_pool(name="ypool", bufs=3))
        # PSUM: scores 2x[128,1024] (4 banks) + pz 2x[65,512] (2 banks)
        #       + projections 2x[128,512] (2 banks) = 8 banks
        scps = ctx.enter_context(tc.tile_pool(name="scps", bufs=2, space="PSUM"))
        pzps = ctx.enter_context(tc.tile_pool(name="pzps", bufs=2, space="PSUM"))
        prps = ctx.enter_context(tc.tile_pool(name="prps", bufs=2, space="PSUM"))

        # ---- resident activations (bf16); allocated in reverse order of
        # death (pool frees are LIFO): xk dies last, xqt first ----
        xk_res, free_xk = tc.tile([128, NDT, SK], BF16, name="xk_res")
        xv_res, free_xv = tc.tile([128, NDT, SK], BF16, name="xv_res")
        xqt_sb, free_xqt = tc.tile([128, NDT, RPC], BF16, name="xqt_sb")
        # DMA priority: xq (Q proj starts first), then xv, then xk.
        nc.sync.dma_start(out=xqt_sb, in_=xqt[:, :].rearrange("(t p) r -> p t r", p=128))
        bq_sb = consts.tile([128, NOT], F32)
        nc.sync.dma_start(out=bq_sb, in_=bq[:, :])
        bk_sb = consts.tile([128, NOT], F32)
        nc.sync.dma_start(out=bk_sb, in_=bk[:, :])
        expb_sb = consts.tile([128, NKT, H], F32)
        nc.sync.dma_start(out=expb_sb, in_=expb[:, :].rearrange("p (kt h) -> p kt h", h=H))
        # kpos-block-major loads so the first K/V projection chunks can
        # start after ~1MB instead of the full 4MB
        for rb in range(NRB):
            for dt in range(NDT):
                nc.scalar.dma_start(
                    out=xv_res[:, dt, 512 * rb:512 * (rb + 1)],
                    in_=xvt[dt * 128:(dt + 1) * 128, 512 * rb:512 * (rb + 1)],
                )
            for dt in range(NDT):
                nc.sync.dma_start(
                    out=xk_res[:, dt, 512 * rb:512 * (rb + 1)],
                    in_=xkt[dt * 128:(dt + 1) * 128, 512 * rb:512 * (rb + 1)],
                )
        eps_sb = consts.tile([128, 1], F32)
        nc.vector.memset(eps_sb, LN_EPS)
        bo_bc = consts.tile([128, ED], BF16)
        g_bc = consts.tile([128, ED], BF16)
        b_bc = consts.tile([128, ED], BF16)

        qT = qzpool.tile([128, NOT, RPC], BF16)  # q^T (pair-major, +bq, /8 folded)
        zT_sb = qzpool.tile([128, NOT, RPC], BF16)  # z^T  [hs, rows]

        # full-batch k^T and v'(+expb col) resident in SBUF (bf16)
        kT_res = kvres.tile([128, NOT, SK], BF16)        # [hs%128, pair, kpos]
        v_res = kvres.tile([128, NKT, H, 65], BF16)      # [kpos%128, kt, head, s|expb]

        # ---- PE warm-up: dummy matmuls on local (memset) data keep the
        # HAM clock-gate at 8/8 while the first input DMAs stream in ----
        warm = consts.tile([128, 128], BF16)
        nc.vector.memset(warm.bitcast(mybir.dt.uint16), 0)
        warmr = consts.tile([128, 512], BF16)
        nc.vector.memset(warmr.bitcast(mybir.dt.uint16), 0)
        for i in range(20):
            wps = prps.tile([128, 512], F32, tag="pr", name=f"warm{i}")
            nc.tensor.matmul(wps, lhsT=warm, rhs=warmr,
                             start=True, stop=True, skip_group_check=True)

        # ---- projection work queue: chunks of ~8 matmuls + evac, drained
        # between attention kt-steps so the PE never starves ----
        pending = []

        # Q projection upfront (all pairs): qT = (x@Wq + bq) / 8
        for tg in range(2):
            wqt = []
            for dt in range(NDT):
                w = wqp.tile([128, 512], BF16, tag="w", name=f"wq{tg}_{dt}")
                nc.gpsimd.dma_start(
                    out=w, in_=wq[dt * 128:(dt + 1) * 128, 512 * tg:512 * (tg + 1)]
                )
                wqt.append(w)
            for tl in range(4):
                t = 4 * tg + tl
                ps = prps.tile([128, RPC], F32, tag="pr", name=f"psq{t}")
                for dt in range(NDT):
                    nc.tensor.matmul(
                        ps, lhsT=wqt[dt][:, 128 * tl:128 * (tl + 1)],
                        rhs=xqt_sb[:, dt, :],
                        start=(dt == 0), stop=(dt == NDT - 1),
                    )
                nc.vector.tensor_scalar(
                    out=qT[:, t, :], in0=ps, scalar1=bq_sb[:, t:t + 1], scalar2=0.125,
                    op0=ALU.add, op1=ALU.mult,
                )

        def queue_k_proj(j, wkt=None):
            """kT_res[:, j, :] = (xk @ Wk_pair_j + bk) as 4 chunks."""
            if wkt is None:
                wkt = []
                for dt in range(NDT):
                    w = wkp.tile([128, 128], BF16, tag="wk", name=f"wk{j}_{dt}")
                    nc.gpsimd.dma_start(
                        out=w, in_=wk[dt * 128:(dt + 1) * 128, 128 * j:128 * (j + 1)]
                    )
                    wkt.append(w)

            def chunk(rb, j=j, wkt=wkt):
                psk = prps.tile([128, 512], F32, tag="pr", name=f"psk{j}_{rb}")
                for dt in range(NDT):
                    nc.tensor.matmul(
                        psk,
                        lhsT=wkt[dt],
                        rhs=xk_res[:, dt, 512 * rb:512 * (rb + 1)],
                        start=(dt == 0), stop=(dt == NDT - 1),
                    )
                nc.vector.tensor_scalar_add(
                    out=kT_res[:, j, 512 * rb:512 * (rb + 1)],
                    in0=psk, scalar1=bk_sb[:, j:j + 1],
                )
            for rb in range(NRB):
                pending.append(lambda rb=rb: chunk(rb))

        def queue_v_proj2(jp, wvt=None):
            """v' for pairs {2jp, 2jp+1} (4 heads) as 16 chunks (per kt)."""
            h0 = 4 * jp
            if wvt is None:
                wvt = []
                for dt in range(NDT):
                    w = wvp.tile([128, 256], BF16, tag="wv", name=f"wv{jp}_{dt}")
                    nc.gpsimd.dma_start(
                        out=w, in_=wv[dt * 128:(dt + 1) * 128, 256 * jp:256 * (jp + 1)]
                    )
                    wvt.append(w)

            def chunk(kt, jp=jp, h0=h0, wvt=wvt):
                psv = prps.tile([128, 256], F32, tag="pr", name=f"psv{jp}_{kt}")
                for dt in range(NDT):
                    nc.tensor.matmul(
                        psv,
                        lhsT=xv_res[:, dt, 128 * kt:128 * (kt + 1)],
                        rhs=wvt[dt],
                        start=(dt == 0), stop=(dt == NDT - 1),
                    )
                nc.vector.tensor_mul(
                    out=v_res[:, kt, h0:h0 + 4, 0:64],
                    in0=psv[:, :].rearrange("p (h c) -> p h c", c=64),
                    in1=expb_sb[:, kt, h0:h0 + 4, None].to_broadcast([128, 4, 64]),
                )
                nc.vector.tensor_copy(
                    out=v_res[:, kt, h0:h0 + 4, 64:65],
                    in_=expb_sb[:, kt, h0:h0 + 4, None],
                )
            for kt in range(NKT):
                pending.append(lambda kt=kt: chunk(kt))

        def drain(n):
            for _ in range(min(n, len(pending))):
                pending.pop(0)()

        def attention(j, dn=1):
            """scores -> exp -> z for head pair j; writes zT_sb[:, j, :]."""
            pz0 = pzps.tile([65, RPC], F32, tag="pz", name=f"pz{j}_0")
            pz1 = pzps.tile([65, RPC], F32, tag="pz", name=f"pz{j}_1")
            pts = [None, None]
            for kt in range(NKT):
                sc = scps.tile([128, 1024], F32, tag="sc", name=f"sc{j}_{kt}")
                nc.tensor.matmul(
                    sc[:, 0:512],
                    lhsT=kT_res[0:64, j, 128 * kt:128 * (kt + 1)],
                    rhs=qT[0:64, j, :],
                    start=True, stop=True,
                )
                nc.tensor.matmul(
                    sc[:, 512:1024],
                    lhsT=kT_res[64:128, j, 128 * kt:128 * (kt + 1)],
                    rhs=qT[64:128, j, :],
                    start=True, stop=True,
                )
                pt = ppool.tile([128, 1024], BF16, tag="p", name=f"pt{j}_{kt}")
                nc.scalar.activation(out=pt, in_=sc, func=AF.Exp)
                pts[kt % 2] = pt
                # proj chunks fill the PE while exp(kt) runs
                drain(dn)
                if kt >= 1:
                    ptp = pts[(kt - 1) % 2]
                    nc.tensor.matmul(
                        pz0, lhsT=v_res[:, kt - 1, 2 * j, :], rhs=ptp[:, 0:512],
                        start=(kt == 1), stop=False, skip_group_check=True,
                    )
                    nc.tensor.matmul(
                        pz1, lhsT=v_res[:, kt - 1, 2 * j + 1, :], rhs=ptp[:, 512:1024],
                        start=(kt == 1), stop=False, skip_group_check=True,
                    )
            ptp = pts[(NKT - 1) % 2]
            nc.tensor.matmul(
                pz0, lhsT=v_res[:, NKT - 1, 2 * j, :], rhs=ptp[:, 0:512],
                start=False, stop=True, skip_group_check=True,
            )
            nc.tensor.matmul(
                pz1, lhsT=v_res[:, NKT - 1, 2 * j + 1, :], rhs=ptp[:, 512:1024],
                start=False, stop=True, skip_group_check=True,
            )
            # normalize: d = pz[64] + 1 (dummy col); z = pz[0:64]*approx(1/d)
            for hh, pz in ((0, pz0), (1, pz1)):
                if j < NOT - 1:
                    # raw copy frees the pz bank fast; the lagging normalize
                    # chain bounces d through DRAM for the partition-broadcast
                    zr = evac.tile([65, RPC], BF16, tag="zr", name=f"zr{j}_{hh}")
                    nc.vector.tensor_copy(out=zr, in_=pz)
                    nc.sync.dma_start(
                        out=dscr[2 * j + hh:2 * j + hh + 1, :], in_=zr[64:65, :]
                    )
                    db_sb = evac.tile([64, RPC], BF16, tag="db", name=f"db{j}_{hh}")
                    nc.sync.dma_start(
                        out=db_sb,
                        in_=dscr[2 * j + hh:2 * j + hh + 1, :].to_broadcast([64, RPC]),
                    )
                    df_sb = evac.tile([64, RPC], F32, tag="df", name=f"df{j}_{hh}")
                    nc.vector.tensor_scalar_add(out=df_sb, in0=db_sb, scalar1=1.0)
                    rb_sb = evac.tile([64, RPC], F32, tag="rb", name=f"rb{j}_{hh}")
                    nc.vector.reciprocal_approx_fast(out=rb_sb, in_=df_sb)
                    zsrc = zr
                else:
                    # last pair sits on the critical path into the out
                    # projection: lowest-latency chain (no DRAM bounce)
                    d_sb = evac.tile([1, RPC], F32, tag="d", name=f"d{j}_{hh}")
                    nc.scalar.activation(out=d_sb, in_=pz[64:65, :],
                                         func=AF.Copy, bias=1.0)
                    df_sb = evac.tile([64, RPC], F32, tag="df", name=f"df{j}_{hh}")
                    nc.gpsimd.partition_broadcast(df_sb, d_sb)
                    rb_sb = evac.tile([64, RPC], F32, tag="rb", name=f"rb{j}_{hh}")
                    nc.vector.reciprocal_approx_fast(out=rb_sb, in_=df_sb)
                    zsrc = pz
                if hh == 0:
                    nc.vector.tensor_mul(
                        out=zT_sb[0:64, j, :], in0=zsrc[0:64, :], in1=rb_sb
                    )
                else:
                    zn = evac.tile([64, RPC], BF16, tag="zn", name=f"zn{j}")
                    nc.vector.tensor_mul(out=zn, in0=zsrc[0:64, :], in1=rb_sb)
                    nc.sync.dma_start(out=zT_sb[64:128, j, :], in_=zn)

        # ---- interleaved pipeline over head pairs ----
        # Q, K(0) and V(0) chunks drain INSIDE attention(0) (dn=2); the
        # kpos-block-major input DMAs make their data available piecewise.
        queue_k_proj(0)
        kq = pending[:]
        del pending[:]
        queue_v_proj2(0)
        vq = pending[:]
        del pending[:]
        for i in range(NRB):
            pending.append(kq[i])
            pending.extend(vq[4 * i:4 * (i + 1)])
        drain(2)  # K(0) rb0 + V kt0 ahead of attention(0)

        wot = None
        yA = None
        for j in range(NOT):
            if j < NOT - 1:
                queue_k_proj(j + 1)
            if j % 2 == 0 and j < NOT - 2:
                queue_v_proj2(j // 2 + 1)  # pairs j+2, j+3
            if j == 3:
                # LN constants, needed from the out-projection onwards
                nc.gpsimd.dma_start(out=bo_bc, in_=bo[:, :].to_broadcast([128, ED]))
                nc.gpsimd.dma_start(out=g_bc, in_=lng[:, :].to_broadcast([128, ED]))
                nc.gpsimd.dma_start(out=b_bc, in_=lnb[:, :].to_broadcast([128, ED]))
            attention(j, dn=2 if j == 0 else 1)
            if j == 1:
                free_xqt()
            if j == NOT - 3:
                free_xv()
            if j == NOT - 2:
                # xk/xv are dead: stage W_O + the bf16 yA accumulator there,
                # then queue out-projection phase A (partials over pairs 0-5)
                # as fill for attention(7)
                free_xk()
                wop = ctx.enter_context(tc.tile_pool(name="wop", bufs=12))
                wot = []
                for jj in range(NOT):
                    w = wop.tile([128, ED], BF16, tag="wo", name=f"wot{jj}")
                    nc.gpsimd.dma_start(out=w, in_=wo[jj * 128:(jj + 1) * 128, :])
                    wot.append(w)
                yA = [wop.tile([128, ED], BF16, tag="ya", name=f"yA{rt}")
                      for rt in range(NRT)]

                def queue_yA(rt, half):
                    def chunk(rt=rt, half=half):
                        psa = prps.tile([128, 512], F32, tag="pr",
                                        name=f"psa{rt}_{half}")
                        for jj in range(NOT - 1):
                            nc.tensor.matmul(
                                psa,
                                lhsT=zT_sb[:, jj, rt * 128:(rt + 1) * 128],
                                rhs=wot[jj][:, 512 * half:512 * (half + 1)],
                                start=(jj == 0), stop=(jj == NOT - 2),
                            )
                        nc.vector.tensor_add(
                            out=yA[rt][:, 512 * half:512 * (half + 1)],
                            in0=psa, in1=bo_bc[:, 512 * half:512 * (half + 1)],
                        )
                    pending.append(chunk)
                for rt in range(NRT):
                    for half in range(2):
                        queue_yA(rt, half)
        drain(100)

        # ---- out projection phase B (last pair) + LayerNorm per row tile ----
        for rt in range(NRT):
            psy = scps.tile([128, 1024], F32, tag="sc", name=f"psy{rt}")
            for half in range(2):
                nc.tensor.matmul(
                    psy[:, 512 * half:512 * (half + 1)],
                    lhsT=zT_sb[:, NOT - 1, rt * 128:(rt + 1) * 128],
                    rhs=wot[NOT - 1][:, 512 * half:512 * (half + 1)],
                    start=True, stop=True,
                )
            y = ypool.tile([128, ED], BF16, tag="y", name=f"y{rt}")
            ysum = evac.tile([128, 1], F32, tag="ys", name=f"ys{rt}")
            nc.vector.scalar_tensor_tensor(
                out=y, in0=psy, scalar=1.0, in1=yA[rt],
                op0=ALU.mult, op1=ALU.add, accum_out=ysum,
            )
            # E[y^2] via ScalarE (free-dim accumulate); the square image is
            # dumped back into the dead psy bank
            sqs = evac.tile([128, 1], F32, tag="qs", name=f"qs{rt}")
            nc.scalar.activation(out=psy, in_=y, func=AF.Square, accum_out=sqs)
            mu = evac.tile([128, 1], F32, tag="mu", name=f"mu{rt}")
            nc.vector.tensor_scalar_mul(out=mu, in0=ysum, scalar1=1.0 / ED)
            mu2 = evac.tile([128, 1], F32, tag="m2", name=f"m2{rt}")
            nc.vector.tensor_mul(out=mu2, in0=mu, in1=mu)
            var = evac.tile([128, 1], F32, tag="vr", name=f"vr{rt}")
            nc.vector.scalar_tensor_tensor(
                out=var, in0=sqs, scalar=1.0 / ED, in1=mu2,
                op0=ALU.mult, op1=ALU.subtract,
            )
            std = evac.tile([128, 1], F32, tag="sd", name=f"sd{rt}")
            nc.scalar.activation(
                out=std, in_=var, func=AF.Sqrt, bias=eps_sb[:, 0:1]
            )
            rstd = evac.tile([128, 1], F32, tag="rs", name=f"rs{rt}")
            nc.vector.reciprocal(out=rstd, in_=std)
            nc.vector.tensor_scalar(
                out=y, in0=y, scalar1=mu, scalar2=rstd,
                op0=ALU.subtract, op1=ALU.mult,
            )
            nc.vector.tensor_mul(out=y, in0=y, in1=g_bc)
            nc.vector.tensor_add(out=y, in0=y, in1=b_bc)
            nc.gpsimd.dma_start(out=out[rt * 128:(rt + 1) * 128, :], in_=y)

    return nc


def prep_in_maps(query, key, value, attention_mask, pos_attn_score,
                 W_Q, b_Q, W_K, b_K, W_V, b_V, W_O, ln_gamma, ln_beta):
    import ml_dtypes
    f32 = np.float32
    bf16 = ml_dtypes.bfloat16
    fp8 = ml_dtypes.float8_e4m3
    q2 = np.asarray(query, f32).reshape(B * SQ, D)
    k2 = np.asarray(key, f32).reshape(B * SK, D)
    v2 = np.asarray(value, f32).reshape(B * SK, D)
    wq2 = np.ascontiguousarray(np.asarray(W_Q, f32).transpose(2, 1, 0).reshape(D, H * HS)).astype(bf16)
    wk2 = np.ascontiguousarray(np.asarray(W_K, f32).transpose(2, 1, 0).reshape(D, H * HS)).astype(bf16)
    wv2 = np.ascontiguousarray(np.asarray(W_V, f32).transpose(2, 1, 0).reshape(D, H * HS)).astype(bf16)
    wo2 = np.ascontiguousarray(np.asarray(W_O, f32).transpose(1, 2, 0).reshape(H * HS, ED)).astype(bf16)
    bq2 = np.ascontiguousarray(np.asarray(b_Q, f32).reshape(NOT, 128).T)
    bk2 = np.ascontiguousarray(np.asarray(b_K, f32).reshape(NOT, 128).T)
    # bo[e] = sum_{h,s} b_V[h,s] * W_O[e,h,s]  (b_V folded past the softmax)
    bo2 = np.einsum("hs,ehs->e", np.asarray(b_V, f32), np.asarray(W_O, f32))
    bo2 = np.ascontiguousarray(bo2.reshape(1, ED))
    pos_np = np.asarray(pos_attn_score, f32)
    mask_np = np.asarray(attention_mask).astype(f32)
    lng = np.ascontiguousarray(np.asarray(ln_gamma, f32).reshape(1, ED)).astype(bf16)
    lnb = np.ascontiguousarray(np.asarray(ln_beta, f32).reshape(1, ED)).astype(bf16)

    kT_by_batch = [np.ascontiguousarray(k2[b * SK:(b + 1) * SK].T).astype(bf16)
                   for b in range(B)]
    vT_by_batch = [np.ascontiguousarray(v2[b * SK:(b + 1) * SK].T).astype(bf16)
                   for b in range(B)]
    # exp_b[k, h] = mask[k] * exp(pos[k, h]), laid out [128, NKT*H]
    expb_by_batch = []
    for b in range(B):
        eb = np.exp(pos_np[b]) * mask_np[b][:, None]  # [SK, H]
        eb = eb.reshape(NKT, 128, H).transpose(1, 0, 2).reshape(128, NKT * H)
        expb_by_batch.append(np.ascontiguousarray(eb.astype(f32)))

    in_maps = []
    for c in range(NCORES):
        b = c // GROUP
        rows = slice(RPC * c, RPC * (c + 1))
        in_maps.append({
            "xqt": np.ascontiguousarray(q2[rows].T).astype(bf16),
            "xkt": kT_by_batch[b],
            "xvt": vT_by_batch[b],
            "wq": wq2, "wk": wk2, "wv": wv2, "wo": wo2,
            "bq": bq2, "bk": bk2, "bo": bo2,
            "expb": expb_by_batch[b],
            "lng": lng, "lnb": lnb,
        })
    return in_maps


def kernel(**inputs):
    global LAST_EXEC_NS
    in_maps = prep_in_maps(**inputs)
    if "nc" not in _CACHED:
        nc = _build()
        nc.finalize()
        _CACHED["nc"] = nc
    nc = _CACHED["nc"]

    trace = bool(os.environ.get("BASS_TRACE"))
    res = run_bass_kernel_spmd(nc, in_maps, core_ids=list(range(NCORES)),
                               trace=trace)
    LAST_EXEC_NS = res.exec_time_ns
    _CACHED["last_result"] = res

    out = np.empty((B * SQ, ED), np.float32)
    for c in range(NCORES):
        out[RPC * c:RPC * (c + 1)] = res.results[c]["out"]
    return out.reshape(B, SQ, ED)
